# revision 8
# baseline (speedup 1.0000x reference)
"""Bahdanau-attention kernel for 8 Trainium2 NeuronCores (Bass/Tile).

Contract: kernel(**inputs) takes the FULL unsharded inputs of
nn_Attention_1580547965036 and returns the full [B, S] softmax output.

  hidden:          [NL=2, B=32, H=1024] fp32
  encoder_outputs: [S=2048, B=32, H=1024] fp32
  attn_w:          [H, 2H] fp32,  attn_b: [H] fp32,  v: [H] fp32

Math:  energy = tanh(concat([hidden[-1]] * S, enc) @ attn_w.T + attn_b)
       scores[b, s] = energy[s, b, :] @ v ;  out = softmax(scores, axis=s)

Strategy (data-parallel over batch, 4 batches per core):
  * The concat matmul splits: enc @ w2.T (big, per-s) + hidden[-1] @ w1.T
    (tiny, broadcast over s — computed once and folded into the tanh bias).
  * Big matmul runs on the PE in float32r (fp32 bits, ~FP22 multiply,
    1 cycle/row at N>=256 — bf16 speed at near-fp32 precision).
  * enc is pre-transposed on the host to [B, H, S] so SBUF tiles load with
    the contraction dim on partitions using contiguous 2 KB DMA rows.
  * v-contraction also runs on the PE via per-batch selector columns
    (v embedded in column b, zeros elsewhere) accumulating into one PSUM
    tile; softmax runs on-chip (DVE reduce/reciprocal + ACT exp).
"""

import os
import sys
from contextlib import ExitStack

import numpy as np

for _p in ("/opt/trn_rl_repo",):
    if os.path.isdir(_p) and _p not in sys.path:
        sys.path.append(_p)

import bass_rust
import concourse.bass as bass
import concourse.mybir as mybir
from concourse import bass_utils
from concourse.tile import TileContext

P = 128
F32 = mybir.dt.float32
F32R = mybir.dt.float32r
AFT = mybir.ActivationFunctionType

N_CORES = 8
S, B, H, NL = 2048, 32, 1024, 2
B_LOCAL = B // N_CORES
S_TILE = 512


# --- workaround: this walrus build allows only one semaphore wait per
# instruction in several encodings ("Too many sync wait commands").  Hoist
# excess waits onto same-engine NoOp instructions placed just before the
# original instruction (engine streams execute in block order, so waiting on
# a preceding NoOp is equivalent).
def _split_excess_waits(nc, max_waits=1):
    counter = 0
    for f in nc.m.functions:
        for bb in f.blocks:
            out = []
            changed = False
            for inst in bb.instructions:
                si = inst.sync_info
                waits = list(si.on_wait or []) if si is not None else []
                if len(waits) > max_waits:
                    for w in waits[:-max_waits]:
                        nop = bass_rust.InstNoOp(name=f"I-waitsplit-{counter}")
                        counter += 1
                        nop.engine = inst.engine
                        nop.sync_info = mybir.SyncInfo(on_wait=[w], on_update=[])
                        nc.register_instruction(nop)
                        out.append(nop)
                    si.on_wait = waits[-max_waits:]
                    changed = True
                out.append(inst)
            if changed:
                bb.instructions = out


def build(nc, tc, B_local=B_LOCAL, S=S, H=H, s_tile=S_TILE):
    KC = H // P       # contraction chunks
    HC = H // P       # output-h chunks
    NS = S // s_tile  # s tiles

    encT = nc.dram_tensor("encT", [B_local, H, S], F32R, kind="ExternalInput")
    w1T = nc.dram_tensor("w1T", [H, H], F32R, kind="ExternalInput")
    w2T = nc.dram_tensor("w2T", [H, H], F32R, kind="ExternalInput")
    hidT = nc.dram_tensor("hidT", [H, B_local], F32R, kind="ExternalInput")
    # host-prepared: attn_b chunks [P, HC] and v selector banks [P, HC*B*B]
    bconst = nc.dram_tensor("bconst", [P, HC], F32, kind="ExternalInput")
    vsel_d = nc.dram_tensor(
        "vsel", [P, HC * B_local * B_local], F32R, kind="ExternalInput"
    )
    out = nc.dram_tensor("out", [B_local, S], F32, kind="ExternalOutput")

    ctx = ExitStack()
    with ctx:
        wpool = ctx.enter_context(tc.tile_pool(name="weights", bufs=1))
        w1pool = ctx.enter_context(tc.tile_pool(name="w1", bufs=1))
        encpool = ctx.enter_context(tc.tile_pool(name="enc", bufs=3))
        thpool = ctx.enter_context(tc.tile_pool(name="tanh", bufs=3))
        spool = ctx.enter_context(tc.tile_pool(name="scores", bufs=1))
        pe_pool = ctx.enter_context(tc.tile_pool(name="psum_e", bufs=2, space="PSUM"))
        ps_pool = ctx.enter_context(tc.tile_pool(name="psum_s", bufs=2, space="PSUM"))
        ph_pool = ctx.enter_context(tc.tile_pool(name="psum_h", bufs=2, space="PSUM"))

        # ---- weights / constants to SBUF ----
        w2_sb = wpool.tile([P, KC * H], F32R)            # [k-part, kc*H + h]
        for kc in range(KC):
            nc.sync.dma_start(
                out=w2_sb[:, kc * H : (kc + 1) * H],
                in_=w2T[kc * P : (kc + 1) * P, :],
            )
        # v selector banks: lhsT for (hc, b) = [k, B_local], column b = v chunk
        v_sel = wpool.tile([P, HC * B_local * B_local], F32R)
        nc.sync.dma_start(out=v_sel[:], in_=vsel_d[:, :])
        battn_sb = wpool.tile([P, HC], F32)
        nc.sync.dma_start(out=battn_sb[:], in_=bconst[:, :])
        w1_sb = w1pool.tile([P, KC * H], F32R)
        for kc in range(KC):
            nc.sync.dma_start(
                out=w1_sb[:, kc * H : (kc + 1) * H],
                in_=w1T[kc * P : (kc + 1) * P, :],
            )
        hid_sb = w1pool.tile([P, KC * B_local], F32R)
        for kc in range(KC):
            nc.sync.dma_start(
                out=hid_sb[:, kc * B_local : (kc + 1) * B_local],
                in_=hidT[kc * P : (kc + 1) * P, :],
            )

        # ---- h_proj + attn_b -> per-(h,b) tanh bias ----
        bias_sb = wpool.tile([P, HC * B_local], F32)     # [h-part, hc*B + b]
        for hc in range(HC):
            ph = ph_pool.tile([P, B_local], F32)
            for kc in range(KC):
                nc.tensor.matmul(
                    ph[:],
                    lhsT=w1_sb[:, kc * H + hc * P : kc * H + (hc + 1) * P],
                    rhs=hid_sb[:, kc * B_local : (kc + 1) * B_local],
                    start=(kc == 0),
                    stop=(kc == KC - 1),
                )
            nc.vector.tensor_scalar_add(
                bias_sb[:, hc * B_local : (hc + 1) * B_local],
                ph[:],
                battn_sb[:, hc : hc + 1],
            )

        # ---- main loop ----
        scores_sb = spool.tile([B_local, S], F32)
        for si in range(NS):
            ps = ps_pool.tile([B_local, s_tile], F32)
            for b in range(B_local):
                enc_sb = encpool.tile([P, KC * s_tile], F32R)
                for kc in range(KC):
                    nc.sync.dma_start(
                        out=enc_sb[:, kc * s_tile : (kc + 1) * s_tile],
                        in_=encT[
                            b, kc * P : (kc + 1) * P, si * s_tile : (si + 1) * s_tile
                        ],
                    )
                for hc in range(HC):
                    pe = pe_pool.tile([P, s_tile], F32)
                    for kc in range(KC):
                        nc.tensor.matmul(
                            pe[:],
                            lhsT=w2_sb[:, kc * H + hc * P : kc * H + (hc + 1) * P],
                            rhs=enc_sb[:, kc * s_tile : (kc + 1) * s_tile],
                            start=(kc == 0),
                            stop=(kc == KC - 1),
                        )
                    th = thpool.tile([P, s_tile], F32R)
                    nc.scalar.activation(
                        th[:],
                        pe[:],
                        AFT.Tanh,
                        bias=bias_sb[:, hc * B_local + b : hc * B_local + b + 1],
                        scale=1.0,
                    )
                    sel = (hc * B_local + b) * B_local
                    nc.tensor.matmul(
                        ps[:],
                        lhsT=v_sel[:, sel : sel + B_local],
                        rhs=th[:],
                        start=(b == 0 and hc == 0),
                        stop=(b == B_local - 1 and hc == HC - 1),
                    )
            nc.vector.tensor_copy(
                scores_sb[:, si * s_tile : (si + 1) * s_tile], ps[:]
            )

        # ---- softmax over S, per batch row ----
        neg_m = spool.tile([B_local, 1], F32)
        nc.vector.reduce_max(
            neg_m[:], scores_sb[:], axis=mybir.AxisListType.X, negate=True
        )
        exp_sb = spool.tile([B_local, S], F32)
        sums = spool.tile([B_local, 1], F32)
        nc.scalar.activation(
            exp_sb[:], scores_sb[:], AFT.Exp,
            bias=neg_m[:], scale=1.0, accum_out=sums[:],
        )
        rinv = spool.tile([B_local, 1], F32)
        nc.vector.reciprocal(rinv[:], sums[:])
        out_sb = spool.tile([B_local, S], F32)
        nc.vector.tensor_scalar_mul(out_sb[:], exp_sb[:], rinv[:])
        nc.sync.dma_start(out=out[:, :], in_=out_sb[:])

    return nc


def _make_core_inputs(hidden, encoder_outputs, attn_w, attn_b, v):
    S_, B_, H_ = encoder_outputs.shape
    B_local = B_ // N_CORES
    hidden = np.asarray(hidden, dtype=np.float32)
    enc = np.asarray(encoder_outputs, dtype=np.float32)
    attn_w = np.asarray(attn_w, dtype=np.float32)
    attn_b = np.asarray(attn_b, dtype=np.float32)
    v = np.asarray(v, dtype=np.float32)

    w1T = np.ascontiguousarray(attn_w[:, :H_].T)
    w2T = np.ascontiguousarray(attn_w[:, H_:].T)
    HC = H_ // P
    bconst = np.ascontiguousarray(attn_b.reshape(HC, P).T)          # [P, HC]
    vsel = np.zeros((P, HC * B_local * B_local), dtype=np.float32)  # [P, sel]
    vchunks = v.reshape(HC, P).T                                    # [P, HC]
    for hc in range(HC):
        for b in range(B_local):
            vsel[:, (hc * B_local + b) * B_local + b] = vchunks[:, hc]
    hid_last = hidden[-1]                                  # [B, H]
    encT = enc.transpose(1, 2, 0)                          # [B, H, S] view
    in_maps = []
    for c in range(N_CORES):
        blo, bhi = c * B_local, (c + 1) * B_local
        in_maps.append({
            "encT": np.ascontiguousarray(encT[blo:bhi]),
            "w1T": w1T,
            "w2T": w2T,
            "hidT": np.ascontiguousarray(hid_last[blo:bhi].T),
            "bconst": bconst,
            "vsel": vsel,
        })
    return in_maps


_CACHE = {}


def _get_nc():
    if "nc" not in _CACHE:
        nc = bass.Bass(
            "TRN2", target_bir_lowering=False, debug=False, num_devices=N_CORES
        )
        with TileContext(nc) as tc:
            build(nc, tc)
        _split_excess_waits(nc)
        _CACHE["nc"] = nc
    return _CACHE["nc"]


def kernel(hidden, encoder_outputs, attn_w, attn_b, v):
    in_maps = _make_core_inputs(hidden, encoder_outputs, attn_w, attn_b, v)
    nc = _get_nc()
    res = bass_utils.run_bass_kernel_spmd(nc, in_maps, list(range(N_CORES)))
    out = np.concatenate(
        [np.asarray(res.results[c]["out"]) for c in range(N_CORES)], axis=0
    )
    return out.astype(np.float32)


if __name__ == "__main__":
    rng = np.random.default_rng(0)
    hidden = rng.standard_normal((NL, B, H), dtype=np.float32)
    enc = rng.standard_normal((S, B, H), dtype=np.float32)
    attn_w = (rng.standard_normal((H, 2 * H), dtype=np.float32) / np.sqrt(2 * H)).astype(
        np.float32
    )
    attn_b = (rng.standard_normal(H, dtype=np.float32) * 0.01).astype(np.float32)
    v = (rng.standard_normal(H, dtype=np.float32) / np.sqrt(H)).astype(np.float32)
    got = kernel(hidden, enc, attn_w, attn_b, v)
    print("out shape:", got.shape, "row sums:", got.sum(axis=1)[:4])


# revision 10
# speedup vs baseline: 1.1829x; 1.1829x over previous
"""Bahdanau-attention kernel for 8 Trainium2 NeuronCores (Bass/Tile).

Contract: kernel(**inputs) takes the FULL unsharded inputs of
nn_Attention_1580547965036 and returns the full [B, S] softmax output.

  hidden:          [NL=2, B=32, H=1024] fp32
  encoder_outputs: [S=2048, B=32, H=1024] fp32
  attn_w:          [H, 2H] fp32,  attn_b: [H] fp32,  v: [H] fp32

Math:  energy = tanh(concat([hidden[-1]] * S, enc) @ attn_w.T + attn_b)
       scores[b, s] = energy[s, b, :] @ v ;  out = softmax(scores, axis=s)

Strategy (data-parallel over batch, 4 batches per core):
  * The concat matmul splits: enc @ w2.T (big, per-s) + hidden[-1] @ w1.T
    (tiny, broadcast over s — computed once and folded into the tanh bias).
  * Big matmul runs on the PE in float32r (fp32 bits, ~FP22 multiply,
    1 cycle/row at N>=256 — bf16 speed at near-fp32 precision).
  * enc is pre-transposed on the host to [B, H, S] so SBUF tiles load with
    the contraction dim on partitions using contiguous 2 KB DMA rows.
  * v-contraction also runs on the PE via per-batch selector columns
    (v embedded in column b, zeros elsewhere) accumulating into one PSUM
    tile; softmax runs on-chip (DVE reduce/reciprocal + ACT exp).
"""

import os
import sys
from contextlib import ExitStack

import numpy as np

for _p in ("/opt/trn_rl_repo",):
    if os.path.isdir(_p) and _p not in sys.path:
        sys.path.append(_p)

import bass_rust
import concourse.bass as bass
import concourse.mybir as mybir
from concourse import bass_utils
from concourse.tile import TileContext

P = 128
F32 = mybir.dt.float32
F32R = mybir.dt.float32r
BF16 = mybir.dt.bfloat16
AFT = mybir.ActivationFunctionType

# Matmul operand dtype for the big energy/score matmuls.
# "f32r": fp32 bits, ~FP22 multiply (rel err ~2e-4, MM ~300 ns measured)
# "bf16": bf16 operands (rel err ~4e-3, faster stream + FWL weight loads)
MM_DTYPE = os.environ.get("ATTN_MM_DTYPE", "f32r")

N_CORES = 8
S, B, H, NL = 2048, 32, 1024, 2
B_LOCAL = B // N_CORES
S_TILE = 512


# --- workaround: this walrus build allows only one semaphore wait per
# instruction in several encodings ("Too many sync wait commands").  Hoist
# excess waits onto same-engine NoOp instructions placed just before the
# original instruction (engine streams execute in block order, so waiting on
# a preceding NoOp is equivalent).
def _split_excess_waits(nc, max_waits=1):
    counter = 0
    for f in nc.m.functions:
        for bb in f.blocks:
            out = []
            changed = False
            for inst in bb.instructions:
                si = inst.sync_info
                waits = list(si.on_wait or []) if si is not None else []
                if len(waits) > max_waits:
                    for w in waits[:-max_waits]:
                        nop = bass_rust.InstNoOp(name=f"I-waitsplit-{counter}")
                        counter += 1
                        nop.engine = inst.engine
                        nop.sync_info = mybir.SyncInfo(on_wait=[w], on_update=[])
                        nc.register_instruction(nop)
                        out.append(nop)
                    si.on_wait = waits[-max_waits:]
                    changed = True
                out.append(inst)
            if changed:
                bb.instructions = out


def build(nc, tc, B_local=B_LOCAL, S=S, H=H, s_tile=S_TILE):
    KC = H // P       # contraction chunks
    HC = H // P       # output-h chunks
    NS = S // s_tile  # s tiles

    MMD = BF16 if MM_DTYPE == "bf16" else F32R
    encT = nc.dram_tensor("encT", [B_local, H, S], MMD, kind="ExternalInput")
    w1T = nc.dram_tensor("w1T", [H, H], F32R, kind="ExternalInput")
    w2T = nc.dram_tensor("w2T", [H, H], MMD, kind="ExternalInput")
    hidT = nc.dram_tensor("hidT", [H, B_local], F32R, kind="ExternalInput")
    # host-prepared: attn_b chunks [P, HC] and v selector banks [P, HC*B*B]
    bconst = nc.dram_tensor("bconst", [P, HC], F32, kind="ExternalInput")
    vsel_d = nc.dram_tensor(
        "vsel", [P, HC * B_local * B_local], MMD, kind="ExternalInput"
    )
    out = nc.dram_tensor("out", [B_local, S], F32, kind="ExternalOutput")

    ctx = ExitStack()
    with ctx:
        wpool = ctx.enter_context(tc.tile_pool(name="weights", bufs=1))
        w1pool = ctx.enter_context(tc.tile_pool(name="w1", bufs=1))
        encpool = ctx.enter_context(tc.tile_pool(name="enc", bufs=3))
        thpool = ctx.enter_context(tc.tile_pool(name="tanh", bufs=3))
        spool = ctx.enter_context(tc.tile_pool(name="scores", bufs=1))
        pe_pool = ctx.enter_context(tc.tile_pool(name="psum_e", bufs=2, space="PSUM"))
        ps_pool = ctx.enter_context(tc.tile_pool(name="psum_s", bufs=2, space="PSUM"))
        ph_pool = ctx.enter_context(tc.tile_pool(name="psum_h", bufs=2, space="PSUM"))

        # ---- weights / constants to SBUF ----
        w2_sb = wpool.tile([P, KC * H], MMD)             # [k-part, kc*H + h]
        for kc in range(KC):
            nc.sync.dma_start(
                out=w2_sb[:, kc * H : (kc + 1) * H],
                in_=w2T[kc * P : (kc + 1) * P, :],
            )
        # v selector banks: lhsT for (hc, b) = [k, B_local], column b = v chunk
        v_sel = wpool.tile([P, HC * B_local * B_local], MMD)
        nc.sync.dma_start(out=v_sel[:], in_=vsel_d[:, :])
        battn_sb = wpool.tile([P, HC], F32)
        nc.sync.dma_start(out=battn_sb[:], in_=bconst[:, :])
        w1_sb = w1pool.tile([P, KC * H], F32R)
        for kc in range(KC):
            nc.sync.dma_start(
                out=w1_sb[:, kc * H : (kc + 1) * H],
                in_=w1T[kc * P : (kc + 1) * P, :],
            )
        hid_sb = w1pool.tile([P, KC * B_local], F32R)
        for kc in range(KC):
            nc.sync.dma_start(
                out=hid_sb[:, kc * B_local : (kc + 1) * B_local],
                in_=hidT[kc * P : (kc + 1) * P, :],
            )

        # ---- h_proj + attn_b -> per-(h,b) tanh bias ----
        bias_sb = wpool.tile([P, HC * B_local], F32)     # [h-part, hc*B + b]
        for hc in range(HC):
            ph = ph_pool.tile([P, B_local], F32)
            for kc in range(KC):
                nc.tensor.matmul(
                    ph[:],
                    lhsT=w1_sb[:, kc * H + hc * P : kc * H + (hc + 1) * P],
                    rhs=hid_sb[:, kc * B_local : (kc + 1) * B_local],
                    start=(kc == 0),
                    stop=(kc == KC - 1),
                )
            nc.vector.tensor_scalar_add(
                bias_sb[:, hc * B_local : (hc + 1) * B_local],
                ph[:],
                battn_sb[:, hc : hc + 1],
            )

        # ---- main loop ----
        scores_sb = spool.tile([B_local, S], F32)
        for si in range(NS):
            ps = ps_pool.tile([B_local, s_tile], F32)
            for b in range(B_local):
                enc_sb = encpool.tile([P, KC * s_tile], MMD)
                for kc in range(KC):
                    nc.sync.dma_start(
                        out=enc_sb[:, kc * s_tile : (kc + 1) * s_tile],
                        in_=encT[
                            b, kc * P : (kc + 1) * P, si * s_tile : (si + 1) * s_tile
                        ],
                    )
                for hc in range(HC):
                    pe = pe_pool.tile([P, s_tile], F32)
                    for kc in range(KC):
                        nc.tensor.matmul(
                            pe[:],
                            lhsT=w2_sb[:, kc * H + hc * P : kc * H + (hc + 1) * P],
                            rhs=enc_sb[:, kc * s_tile : (kc + 1) * s_tile],
                            start=(kc == 0),
                            stop=(kc == KC - 1),
                        )
                    th = thpool.tile([P, s_tile], MMD)
                    nc.scalar.activation(
                        th[:],
                        pe[:],
                        AFT.Tanh,
                        bias=bias_sb[:, hc * B_local + b : hc * B_local + b + 1],
                        scale=1.0,
                    )
                    sel = (hc * B_local + b) * B_local
                    nc.tensor.matmul(
                        ps[:],
                        lhsT=v_sel[:, sel : sel + B_local],
                        rhs=th[:],
                        start=(b == 0 and hc == 0),
                        stop=(b == B_local - 1 and hc == HC - 1),
                    )
            nc.vector.tensor_copy(
                scores_sb[:, si * s_tile : (si + 1) * s_tile], ps[:]
            )

        # ---- softmax over S, per batch row ----
        neg_m = spool.tile([B_local, 1], F32)
        nc.vector.reduce_max(
            neg_m[:], scores_sb[:], axis=mybir.AxisListType.X, negate=True
        )
        exp_sb = spool.tile([B_local, S], F32)
        sums = spool.tile([B_local, 1], F32)
        nc.scalar.activation(
            exp_sb[:], scores_sb[:], AFT.Exp,
            bias=neg_m[:], scale=1.0, accum_out=sums[:],
        )
        rinv = spool.tile([B_local, 1], F32)
        nc.vector.reciprocal(rinv[:], sums[:])
        out_sb = spool.tile([B_local, S], F32)
        nc.vector.tensor_scalar_mul(out_sb[:], exp_sb[:], rinv[:])
        nc.sync.dma_start(out=out[:, :], in_=out_sb[:])

    return nc


def _make_core_inputs(hidden, encoder_outputs, attn_w, attn_b, v):
    S_, B_, H_ = encoder_outputs.shape
    B_local = B_ // N_CORES
    hidden = np.asarray(hidden, dtype=np.float32)
    enc = np.asarray(encoder_outputs, dtype=np.float32)
    attn_w = np.asarray(attn_w, dtype=np.float32)
    attn_b = np.asarray(attn_b, dtype=np.float32)
    v = np.asarray(v, dtype=np.float32)

    import ml_dtypes
    mmd_np = ml_dtypes.bfloat16 if MM_DTYPE == "bf16" else np.float32
    w1T = np.ascontiguousarray(attn_w[:, :H_].T)
    w2T = np.ascontiguousarray(attn_w[:, H_:].T).astype(mmd_np)
    HC = H_ // P
    bconst = np.ascontiguousarray(attn_b.reshape(HC, P).T)          # [P, HC]
    vsel = np.zeros((P, HC * B_local * B_local), dtype=mmd_np)      # [P, sel]
    vchunks = v.reshape(HC, P).T                                    # [P, HC]
    for hc in range(HC):
        for b in range(B_local):
            vsel[:, (hc * B_local + b) * B_local + b] = vchunks[:, hc].astype(mmd_np)
    hid_last = hidden[-1]                                  # [B, H]
    encT = enc.transpose(1, 2, 0)                          # [B, H, S] view
    in_maps = []
    for c in range(N_CORES):
        blo, bhi = c * B_local, (c + 1) * B_local
        in_maps.append({
            "encT": np.ascontiguousarray(encT[blo:bhi]).astype(mmd_np),
            "w1T": w1T,
            "w2T": w2T,
            "hidT": np.ascontiguousarray(hid_last[blo:bhi].T),
            "bconst": bconst,
            "vsel": vsel,
        })
    return in_maps


_CACHE = {}


def _get_nc():
    if "nc" not in _CACHE:
        nc = bass.Bass(
            "TRN2", target_bir_lowering=False, debug=False, num_devices=N_CORES
        )
        with TileContext(nc) as tc:
            build(nc, tc)
        _split_excess_waits(nc)
        _CACHE["nc"] = nc
    return _CACHE["nc"]


def kernel(hidden, encoder_outputs, attn_w, attn_b, v):
    in_maps = _make_core_inputs(hidden, encoder_outputs, attn_w, attn_b, v)
    nc = _get_nc()
    res = bass_utils.run_bass_kernel_spmd(nc, in_maps, list(range(N_CORES)))
    out = np.concatenate(
        [np.asarray(res.results[c]["out"]) for c in range(N_CORES)], axis=0
    )
    return out.astype(np.float32)


if __name__ == "__main__":
    rng = np.random.default_rng(0)
    hidden = rng.standard_normal((NL, B, H), dtype=np.float32)
    enc = rng.standard_normal((S, B, H), dtype=np.float32)
    attn_w = (rng.standard_normal((H, 2 * H), dtype=np.float32) / np.sqrt(2 * H)).astype(
        np.float32
    )
    attn_b = (rng.standard_normal(H, dtype=np.float32) * 0.01).astype(np.float32)
    v = (rng.standard_normal(H, dtype=np.float32) / np.sqrt(H)).astype(np.float32)
    got = kernel(hidden, enc, attn_w, attn_b, v)
    print("out shape:", got.shape, "row sums:", got.sum(axis=1)[:4])


# revision 11
# speedup vs baseline: 1.2394x; 1.0478x over previous
"""Bahdanau-attention kernel for 8 Trainium2 NeuronCores (Bass/Tile).

Contract: kernel(**inputs) takes the FULL unsharded inputs of
nn_Attention_1580547965036 and returns the full [B, S] softmax output.

  hidden:          [NL=2, B=32, H=1024] fp32
  encoder_outputs: [S=2048, B=32, H=1024] fp32
  attn_w:          [H, 2H] fp32,  attn_b: [H] fp32,  v: [H] fp32

Math:  energy = tanh(concat([hidden[-1]] * S, enc) @ attn_w.T + attn_b)
       scores[b, s] = energy[s, b, :] @ v ;  out = softmax(scores, axis=s)

Strategy (data-parallel over batch, 4 batches per core):
  * The concat matmul splits: enc @ w2.T (big, per-s) + hidden[-1] @ w1.T
    (tiny, broadcast over s — computed once and folded into the tanh bias).
  * Big matmul runs on the PE in float32r (fp32 bits, ~FP22 multiply,
    1 cycle/row at N>=256 — bf16 speed at near-fp32 precision).
  * enc is pre-transposed on the host to [B, H, S] so SBUF tiles load with
    the contraction dim on partitions using contiguous 2 KB DMA rows.
  * v-contraction also runs on the PE via per-batch selector columns
    (v embedded in column b, zeros elsewhere) accumulating into one PSUM
    tile; softmax runs on-chip (DVE reduce/reciprocal + ACT exp).
"""

import os
import sys
from contextlib import ExitStack

import numpy as np

for _p in ("/opt/trn_rl_repo",):
    if os.path.isdir(_p) and _p not in sys.path:
        sys.path.append(_p)

import bass_rust
import concourse.bass as bass
import concourse.mybir as mybir
from concourse import bass_utils
from concourse.tile import TileContext

P = 128
F32 = mybir.dt.float32
F32R = mybir.dt.float32r
BF16 = mybir.dt.bfloat16
AFT = mybir.ActivationFunctionType

# Matmul operand dtype for the big energy/score matmuls.
# "f32r": fp32 bits, ~FP22 multiply (rel err ~2e-4, MM ~300 ns measured)
# "bf16": bf16 operands (rel err ~4e-3, faster stream + FWL weight loads)
MM_DTYPE = os.environ.get("ATTN_MM_DTYPE", "f32r")

N_CORES = 8
S, B, H, NL = 2048, 32, 1024, 2
B_LOCAL = B // N_CORES
S_TILE = 512


# --- workaround: this walrus build allows only one semaphore wait per
# instruction in several encodings ("Too many sync wait commands").  Hoist
# excess waits onto same-engine NoOp instructions placed just before the
# original instruction (engine streams execute in block order, so waiting on
# a preceding NoOp is equivalent).
def _split_excess_waits(nc, max_waits=1):
    counter = 0
    for f in nc.m.functions:
        for bb in f.blocks:
            out = []
            changed = False
            for inst in bb.instructions:
                si = inst.sync_info
                waits = list(si.on_wait or []) if si is not None else []
                if len(waits) > max_waits:
                    for w in waits[:-max_waits]:
                        nop = bass_rust.InstNoOp(name=f"I-waitsplit-{counter}")
                        counter += 1
                        nop.engine = inst.engine
                        nop.sync_info = mybir.SyncInfo(on_wait=[w], on_update=[])
                        nc.register_instruction(nop)
                        out.append(nop)
                    si.on_wait = waits[-max_waits:]
                    changed = True
                out.append(inst)
            if changed:
                bb.instructions = out


def build(nc, tc, B_local=B_LOCAL, S=S, H=H, s_tile=S_TILE):
    KC = H // P       # contraction chunks
    HC = H // P       # output-h chunks
    NS = S // s_tile  # s tiles

    MMD = BF16 if MM_DTYPE == "bf16" else F32R
    encT = nc.dram_tensor("encT", [B_local, H, S], MMD, kind="ExternalInput")
    w1T = nc.dram_tensor("w1T", [H, H], MMD, kind="ExternalInput")
    w2T = nc.dram_tensor("w2T", [H, H], MMD, kind="ExternalInput")
    hidT = nc.dram_tensor("hidT", [H, B_local], MMD, kind="ExternalInput")
    # host-prepared: attn_b chunks [P, HC] and v selector banks [P, HC*B*B]
    bconst = nc.dram_tensor("bconst", [P, HC], F32, kind="ExternalInput")
    vsel_d = nc.dram_tensor(
        "vsel", [P, HC * B_local * B_local], MMD, kind="ExternalInput"
    )
    out = nc.dram_tensor("out", [B_local, S], F32, kind="ExternalOutput")

    ctx = ExitStack()
    with ctx:
        wpool = ctx.enter_context(tc.tile_pool(name="weights", bufs=1))
        w1pool = ctx.enter_context(tc.tile_pool(name="w1", bufs=1))
        encpool = ctx.enter_context(tc.tile_pool(name="enc", bufs=3))
        thpool = ctx.enter_context(tc.tile_pool(name="tanh", bufs=4))
        spool = ctx.enter_context(tc.tile_pool(name="scores", bufs=1))
        pe_pool = ctx.enter_context(tc.tile_pool(name="psum_e", bufs=4, space="PSUM"))
        ps_pool = ctx.enter_context(tc.tile_pool(name="psum_s", bufs=2, space="PSUM"))
        ph_pool = ctx.enter_context(tc.tile_pool(name="psum_h", bufs=1, space="PSUM"))

        # ---- weights / constants to SBUF (w1/hid first: h_proj gates the
        # first tanh, so its inputs should land before the bulk w2 bytes) ----
        w1_sb = w1pool.tile([P, KC * H], MMD)
        for kc in range(KC):
            nc.sync.dma_start(
                out=w1_sb[:, kc * H : (kc + 1) * H],
                in_=w1T[kc * P : (kc + 1) * P, :],
            )
        hid_sb = w1pool.tile([P, KC * B_local], MMD)
        for kc in range(KC):
            nc.sync.dma_start(
                out=hid_sb[:, kc * B_local : (kc + 1) * B_local],
                in_=hidT[kc * P : (kc + 1) * P, :],
            )
        battn_sb = wpool.tile([P, HC], F32)
        nc.sync.dma_start(out=battn_sb[:], in_=bconst[:, :])
        v_sel = wpool.tile([P, HC * B_local * B_local], MMD)
        nc.sync.dma_start(out=v_sel[:], in_=vsel_d[:, :])
        w2_sb = wpool.tile([P, KC * H], MMD)             # [k-part, kc*H + h]
        for kc in range(KC):
            nc.sync.dma_start(
                out=w2_sb[:, kc * H : (kc + 1) * H],
                in_=w2T[kc * P : (kc + 1) * P, :],
            )

        # ---- h_proj + attn_b -> per-(h,b) tanh bias ----
        bias_sb = wpool.tile([P, HC * B_local], F32)     # [h-part, hc*B + b]
        for hc in range(HC):
            ph = ph_pool.tile([P, B_local], F32)
            for kc in range(KC):
                nc.tensor.matmul(
                    ph[:],
                    lhsT=w1_sb[:, kc * H + hc * P : kc * H + (hc + 1) * P],
                    rhs=hid_sb[:, kc * B_local : (kc + 1) * B_local],
                    start=(kc == 0),
                    stop=(kc == KC - 1),
                )
            nc.vector.tensor_scalar_add(
                bias_sb[:, hc * B_local : (hc + 1) * B_local],
                ph[:],
                battn_sb[:, hc : hc + 1],
            )

        # ---- main loop ----
        scores_sb = spool.tile([B_local, S], F32)
        for si in range(NS):
            ps = ps_pool.tile([B_local, s_tile], F32)
            for b in range(B_local):
                enc_sb = encpool.tile([P, KC * s_tile], MMD)
                for kc in range(KC):
                    nc.sync.dma_start(
                        out=enc_sb[:, kc * s_tile : (kc + 1) * s_tile],
                        in_=encT[
                            b, kc * P : (kc + 1) * P, si * s_tile : (si + 1) * s_tile
                        ],
                    )
                for hc in range(HC):
                    pe = pe_pool.tile([P, s_tile], F32)
                    for kc in range(KC):
                        nc.tensor.matmul(
                            pe[:],
                            lhsT=w2_sb[:, kc * H + hc * P : kc * H + (hc + 1) * P],
                            rhs=enc_sb[:, kc * s_tile : (kc + 1) * s_tile],
                            start=(kc == 0),
                            stop=(kc == KC - 1),
                        )
                    th = thpool.tile([P, s_tile], MMD)
                    nc.scalar.activation(
                        th[:],
                        pe[:],
                        AFT.Tanh,
                        bias=bias_sb[:, hc * B_local + b : hc * B_local + b + 1],
                        scale=1.0,
                    )
                    sel = (hc * B_local + b) * B_local
                    nc.tensor.matmul(
                        ps[:],
                        lhsT=v_sel[:, sel : sel + B_local],
                        rhs=th[:],
                        start=(b == 0 and hc == 0),
                        stop=(b == B_local - 1 and hc == HC - 1),
                    )
            nc.vector.tensor_copy(
                scores_sb[:, si * s_tile : (si + 1) * s_tile], ps[:]
            )

        # ---- softmax over S, per batch row ----
        neg_m = spool.tile([B_local, 1], F32)
        nc.vector.reduce_max(
            neg_m[:], scores_sb[:], axis=mybir.AxisListType.X, negate=True
        )
        exp_sb = spool.tile([B_local, S], F32)
        sums = spool.tile([B_local, 1], F32)
        nc.scalar.activation(
            exp_sb[:], scores_sb[:], AFT.Exp,
            bias=neg_m[:], scale=1.0, accum_out=sums[:],
        )
        rinv = spool.tile([B_local, 1], F32)
        nc.vector.reciprocal(rinv[:], sums[:])
        out_sb = spool.tile([B_local, S], F32)
        nc.vector.tensor_scalar_mul(out_sb[:], exp_sb[:], rinv[:])
        nc.sync.dma_start(out=out[:, :], in_=out_sb[:])

    return nc


def _make_core_inputs(hidden, encoder_outputs, attn_w, attn_b, v):
    S_, B_, H_ = encoder_outputs.shape
    B_local = B_ // N_CORES
    hidden = np.asarray(hidden, dtype=np.float32)
    enc = np.asarray(encoder_outputs, dtype=np.float32)
    attn_w = np.asarray(attn_w, dtype=np.float32)
    attn_b = np.asarray(attn_b, dtype=np.float32)
    v = np.asarray(v, dtype=np.float32)

    import ml_dtypes
    mmd_np = ml_dtypes.bfloat16 if MM_DTYPE == "bf16" else np.float32
    w1T = np.ascontiguousarray(attn_w[:, :H_].T).astype(mmd_np)
    w2T = np.ascontiguousarray(attn_w[:, H_:].T).astype(mmd_np)
    HC = H_ // P
    bconst = np.ascontiguousarray(attn_b.reshape(HC, P).T)          # [P, HC]
    vsel = np.zeros((P, HC * B_local * B_local), dtype=mmd_np)      # [P, sel]
    vchunks = v.reshape(HC, P).T                                    # [P, HC]
    for hc in range(HC):
        for b in range(B_local):
            vsel[:, (hc * B_local + b) * B_local + b] = vchunks[:, hc].astype(mmd_np)
    hid_last = hidden[-1]                                  # [B, H]
    encT = enc.transpose(1, 2, 0)                          # [B, H, S] view
    in_maps = []
    for c in range(N_CORES):
        blo, bhi = c * B_local, (c + 1) * B_local
        in_maps.append({
            "encT": np.ascontiguousarray(encT[blo:bhi]).astype(mmd_np),
            "w1T": w1T,
            "w2T": w2T,
            "hidT": np.ascontiguousarray(hid_last[blo:bhi].T).astype(mmd_np),
            "bconst": bconst,
            "vsel": vsel,
        })
    return in_maps


_CACHE = {}


def _get_nc():
    if "nc" not in _CACHE:
        nc = bass.Bass(
            "TRN2", target_bir_lowering=False, debug=False, num_devices=N_CORES
        )
        with TileContext(nc) as tc:
            build(nc, tc)
        _split_excess_waits(nc)
        _CACHE["nc"] = nc
    return _CACHE["nc"]


def kernel(hidden, encoder_outputs, attn_w, attn_b, v):
    in_maps = _make_core_inputs(hidden, encoder_outputs, attn_w, attn_b, v)
    nc = _get_nc()
    res = bass_utils.run_bass_kernel_spmd(nc, in_maps, list(range(N_CORES)))
    out = np.concatenate(
        [np.asarray(res.results[c]["out"]) for c in range(N_CORES)], axis=0
    )
    return out.astype(np.float32)


if __name__ == "__main__":
    rng = np.random.default_rng(0)
    hidden = rng.standard_normal((NL, B, H), dtype=np.float32)
    enc = rng.standard_normal((S, B, H), dtype=np.float32)
    attn_w = (rng.standard_normal((H, 2 * H), dtype=np.float32) / np.sqrt(2 * H)).astype(
        np.float32
    )
    attn_b = (rng.standard_normal(H, dtype=np.float32) * 0.01).astype(np.float32)
    v = (rng.standard_normal(H, dtype=np.float32) / np.sqrt(H)).astype(np.float32)
    got = kernel(hidden, enc, attn_w, attn_b, v)
    print("out shape:", got.shape, "row sums:", got.sum(axis=1)[:4])


# revision 12
# speedup vs baseline: 1.2642x; 1.0200x over previous
"""Bahdanau-attention kernel for 8 Trainium2 NeuronCores (Bass/Tile).

Contract: kernel(**inputs) takes the FULL unsharded inputs of
nn_Attention_1580547965036 and returns the full [B, S] softmax output.

  hidden:          [NL=2, B=32, H=1024] fp32
  encoder_outputs: [S=2048, B=32, H=1024] fp32
  attn_w:          [H, 2H] fp32,  attn_b: [H] fp32,  v: [H] fp32

Math:  energy = tanh(concat([hidden[-1]] * S, enc) @ attn_w.T + attn_b)
       scores[b, s] = energy[s, b, :] @ v ;  out = softmax(scores, axis=s)

Strategy (data-parallel over batch, 4 batches per core):
  * The concat matmul splits: enc @ w2.T (big, per-s) + hidden[-1] @ w1.T
    (tiny, broadcast over s — computed once and folded into the tanh bias).
  * Big matmul runs on the PE in float32r (fp32 bits, ~FP22 multiply,
    1 cycle/row at N>=256 — bf16 speed at near-fp32 precision).
  * enc is pre-transposed on the host to [B, H, S] so SBUF tiles load with
    the contraction dim on partitions using contiguous 2 KB DMA rows.
  * v-contraction also runs on the PE via per-batch selector columns
    (v embedded in column b, zeros elsewhere) accumulating into one PSUM
    tile; softmax runs on-chip (DVE reduce/reciprocal + ACT exp).
"""

import os
import sys
from contextlib import ExitStack

import numpy as np

for _p in ("/opt/trn_rl_repo",):
    if os.path.isdir(_p) and _p not in sys.path:
        sys.path.append(_p)

import bass_rust
import concourse.bass as bass
import concourse.mybir as mybir
from concourse import bass_utils
from concourse.tile import TileContext

P = 128
F32 = mybir.dt.float32
F32R = mybir.dt.float32r
BF16 = mybir.dt.bfloat16
AFT = mybir.ActivationFunctionType

# Matmul operand dtype for the big energy/score matmuls.
# "f32r": fp32 bits, ~FP22 multiply (rel err ~2e-4, MM ~300 ns measured)
# "bf16": bf16 operands (rel err ~4e-3, faster stream + FWL weight loads)
MM_DTYPE = os.environ.get("ATTN_MM_DTYPE", "f32r")

N_CORES = 8
S, B, H, NL = 2048, 32, 1024, 2
B_LOCAL = B // N_CORES
S_TILE = 512


# --- workaround: this walrus build allows only one semaphore wait per
# instruction in several encodings ("Too many sync wait commands").  Hoist
# excess waits onto same-engine NoOp instructions placed just before the
# original instruction (engine streams execute in block order, so waiting on
# a preceding NoOp is equivalent).
def _split_excess_waits(nc, max_waits=1):
    counter = 0
    for f in nc.m.functions:
        for bb in f.blocks:
            out = []
            changed = False
            for inst in bb.instructions:
                si = inst.sync_info
                waits = list(si.on_wait or []) if si is not None else []
                if len(waits) > max_waits:
                    for w in waits[:-max_waits]:
                        nop = bass_rust.InstNoOp(name=f"I-waitsplit-{counter}")
                        counter += 1
                        nop.engine = inst.engine
                        nop.sync_info = mybir.SyncInfo(on_wait=[w], on_update=[])
                        nc.register_instruction(nop)
                        out.append(nop)
                    si.on_wait = waits[-max_waits:]
                    changed = True
                out.append(inst)
            if changed:
                bb.instructions = out


def build(nc, tc, B_local=B_LOCAL, S=S, H=H, s_tile=S_TILE):
    KC = H // P       # contraction chunks
    HC = H // P       # output-h chunks
    NS = S // s_tile  # s tiles

    MMD = BF16 if MM_DTYPE == "bf16" else F32R
    encT = nc.dram_tensor("encT", [B_local, H, S], MMD, kind="ExternalInput")
    w1T = nc.dram_tensor("w1T", [H, H], MMD, kind="ExternalInput")
    w2T = nc.dram_tensor("w2T", [H, H], MMD, kind="ExternalInput")
    hidT = nc.dram_tensor("hidT", [H, B_local], MMD, kind="ExternalInput")
    # host-prepared: attn_b chunks [P, HC] and v selector banks [P, HC*B*B]
    bconst = nc.dram_tensor("bconst", [P, HC], F32, kind="ExternalInput")
    vsel_d = nc.dram_tensor(
        "vsel", [P, HC * B_local * B_local], MMD, kind="ExternalInput"
    )
    out = nc.dram_tensor("out", [B_local, S], F32, kind="ExternalOutput")

    ctx = ExitStack()
    with ctx:
        wpool = ctx.enter_context(tc.tile_pool(name="weights", bufs=1))
        w1pool = ctx.enter_context(tc.tile_pool(name="w1", bufs=1))
        encpool = ctx.enter_context(tc.tile_pool(name="enc", bufs=4))
        thpool = ctx.enter_context(tc.tile_pool(name="tanh", bufs=4))
        spool = ctx.enter_context(tc.tile_pool(name="scores", bufs=1))
        pe_pool = ctx.enter_context(tc.tile_pool(name="psum_e", bufs=4, space="PSUM"))
        ps_pool = ctx.enter_context(tc.tile_pool(name="psum_s", bufs=2, space="PSUM"))
        ph_pool = ctx.enter_context(tc.tile_pool(name="psum_h", bufs=1, space="PSUM"))

        def load_enc(si, b):
            t = encpool.tile([P, KC * s_tile], MMD)
            nc.sync.dma_start(
                out=t[:].rearrange("p (kc s) -> p kc s", kc=KC),
                in_=encT[b].rearrange("(kc p) s -> p kc s", p=P)[
                    :, :, si * s_tile : (si + 1) * s_tile
                ],
            )
            return t

        # first enc tile queued before the weight bytes so the PE can start
        # energy matmuls as early as possible
        enc0 = load_enc(0, 0)

        w1_sb = w1pool.tile([P, KC * H], MMD)
        nc.sync.dma_start(
            out=w1_sb[:].rearrange("p (kc h) -> p kc h", kc=KC),
            in_=w1T.rearrange("(kc p) h -> p kc h", p=P),
        )
        hid_sb = w1pool.tile([P, KC * B_local], MMD)
        nc.sync.dma_start(
            out=hid_sb[:].rearrange("p (kc b) -> p kc b", kc=KC),
            in_=hidT.rearrange("(kc p) b -> p kc b", p=P),
        )
        battn_sb = wpool.tile([P, HC], F32)
        nc.sync.dma_start(out=battn_sb[:], in_=bconst[:, :])
        v_sel = wpool.tile([P, HC * B_local * B_local], MMD)
        nc.sync.dma_start(out=v_sel[:], in_=vsel_d[:, :])
        w2_sb = wpool.tile([P, KC * H], MMD)             # [k-part, kc*H + h]
        nc.sync.dma_start(
            out=w2_sb[:].rearrange("p (kc h) -> p kc h", kc=KC),
            in_=w2T.rearrange("(kc p) h -> p kc h", p=P),
        )

        # ---- h_proj + attn_b -> per-(h,b) tanh bias ----
        bias_sb = wpool.tile([P, HC * B_local], F32)     # [h-part, hc*B + b]
        for hc in range(HC):
            ph = ph_pool.tile([P, B_local], F32)
            for kc in range(KC):
                nc.tensor.matmul(
                    ph[:],
                    lhsT=w1_sb[:, kc * H + hc * P : kc * H + (hc + 1) * P],
                    rhs=hid_sb[:, kc * B_local : (kc + 1) * B_local],
                    start=(kc == 0),
                    stop=(kc == KC - 1),
                )
            nc.vector.tensor_scalar_add(
                bias_sb[:, hc * B_local : (hc + 1) * B_local],
                ph[:],
                battn_sb[:, hc : hc + 1],
            )

        # ---- main loop ----
        scores_sb = spool.tile([B_local, S], F32)
        for si in range(NS):
            ps = ps_pool.tile([B_local, s_tile], F32)
            for b in range(B_local):
                enc_sb = enc0 if (si == 0 and b == 0) else load_enc(si, b)
                pending_v = None  # (sel_col, tanh_tile), emitted one group late
                for hc in range(HC):
                    pe = pe_pool.tile([P, s_tile], F32)
                    for kc in range(KC):
                        nc.tensor.matmul(
                            pe[:],
                            lhsT=w2_sb[:, kc * H + hc * P : kc * H + (hc + 1) * P],
                            rhs=enc_sb[:, kc * s_tile : (kc + 1) * s_tile],
                            start=(kc == 0),
                            stop=(kc == KC - 1),
                        )
                    th = thpool.tile([P, s_tile], MMD)
                    nc.scalar.activation(
                        th[:],
                        pe[:],
                        AFT.Tanh,
                        bias=bias_sb[:, hc * B_local + b : hc * B_local + b + 1],
                        scale=1.0,
                    )
                    if pending_v is not None:
                        sel, pth, first = pending_v
                        nc.tensor.matmul(
                            ps[:], lhsT=v_sel[:, sel : sel + B_local], rhs=pth[:],
                            start=first, stop=False,
                        )
                    pending_v = (
                        (hc * B_local + b) * B_local,
                        th,
                        (b == 0 and hc == 0),
                    )
                sel, pth, first = pending_v
                nc.tensor.matmul(
                    ps[:], lhsT=v_sel[:, sel : sel + B_local], rhs=pth[:],
                    start=first, stop=(b == B_local - 1),
                )
            nc.vector.tensor_copy(
                scores_sb[:, si * s_tile : (si + 1) * s_tile], ps[:]
            )

        # ---- softmax over S, per batch row (scores are O(1); exp is safe
        # in fp32 without the max subtraction) ----
        exp_sb = spool.tile([B_local, S], F32)
        sums = spool.tile([B_local, 1], F32)
        nc.scalar.activation(
            exp_sb[:], scores_sb[:], AFT.Exp, scale=1.0, accum_out=sums[:],
        )
        rinv = spool.tile([B_local, 1], F32)
        nc.vector.reciprocal(rinv[:], sums[:])
        out_sb = spool.tile([B_local, S], F32)
        nc.vector.tensor_scalar_mul(out_sb[:], exp_sb[:], rinv[:])
        nc.sync.dma_start(out=out[:, :], in_=out_sb[:])

    return nc


def _make_core_inputs(hidden, encoder_outputs, attn_w, attn_b, v):
    S_, B_, H_ = encoder_outputs.shape
    B_local = B_ // N_CORES
    hidden = np.asarray(hidden, dtype=np.float32)
    enc = np.asarray(encoder_outputs, dtype=np.float32)
    attn_w = np.asarray(attn_w, dtype=np.float32)
    attn_b = np.asarray(attn_b, dtype=np.float32)
    v = np.asarray(v, dtype=np.float32)

    import ml_dtypes
    mmd_np = ml_dtypes.bfloat16 if MM_DTYPE == "bf16" else np.float32
    w1T = np.ascontiguousarray(attn_w[:, :H_].T).astype(mmd_np)
    w2T = np.ascontiguousarray(attn_w[:, H_:].T).astype(mmd_np)
    HC = H_ // P
    bconst = np.ascontiguousarray(attn_b.reshape(HC, P).T)          # [P, HC]
    vsel = np.zeros((P, HC * B_local * B_local), dtype=mmd_np)      # [P, sel]
    vchunks = v.reshape(HC, P).T                                    # [P, HC]
    for hc in range(HC):
        for b in range(B_local):
            vsel[:, (hc * B_local + b) * B_local + b] = vchunks[:, hc].astype(mmd_np)
    hid_last = hidden[-1]                                  # [B, H]
    encT = enc.transpose(1, 2, 0)                          # [B, H, S] view
    in_maps = []
    for c in range(N_CORES):
        blo, bhi = c * B_local, (c + 1) * B_local
        in_maps.append({
            "encT": np.ascontiguousarray(encT[blo:bhi]).astype(mmd_np),
            "w1T": w1T,
            "w2T": w2T,
            "hidT": np.ascontiguousarray(hid_last[blo:bhi].T).astype(mmd_np),
            "bconst": bconst,
            "vsel": vsel,
        })
    return in_maps


_CACHE = {}


def _get_nc():
    if "nc" not in _CACHE:
        nc = bass.Bass(
            "TRN2", target_bir_lowering=False, debug=False, num_devices=N_CORES
        )
        with TileContext(nc) as tc:
            build(nc, tc)
        _split_excess_waits(nc)
        _CACHE["nc"] = nc
    return _CACHE["nc"]


def kernel(hidden, encoder_outputs, attn_w, attn_b, v):
    in_maps = _make_core_inputs(hidden, encoder_outputs, attn_w, attn_b, v)
    nc = _get_nc()
    res = bass_utils.run_bass_kernel_spmd(nc, in_maps, list(range(N_CORES)))
    out = np.concatenate(
        [np.asarray(res.results[c]["out"]) for c in range(N_CORES)], axis=0
    )
    return out.astype(np.float32)


if __name__ == "__main__":
    rng = np.random.default_rng(0)
    hidden = rng.standard_normal((NL, B, H), dtype=np.float32)
    enc = rng.standard_normal((S, B, H), dtype=np.float32)
    attn_w = (rng.standard_normal((H, 2 * H), dtype=np.float32) / np.sqrt(2 * H)).astype(
        np.float32
    )
    attn_b = (rng.standard_normal(H, dtype=np.float32) * 0.01).astype(np.float32)
    v = (rng.standard_normal(H, dtype=np.float32) / np.sqrt(H)).astype(np.float32)
    got = kernel(hidden, enc, attn_w, attn_b, v)
    print("out shape:", got.shape, "row sums:", got.sum(axis=1)[:4])
